# revision 35
# baseline (speedup 1.0000x reference)
"""AttentionalPooler Trainium2 kernel.

Full inputs -> full output; batch (8) is data-parallel across the 8
NeuronCores. Per core: LayerNorm(x_b), kv = LN(x_b) @ Wkv, 12-head
cross-attention from 256 pre-computed queries, output projection.

Host-side preprocessing (exact fp32 algebra, batch-independent):
  - q path (LN(query) @ Wq * dh^-0.5, transposed) is computed on host.
  - ln_k_w is folded into the kv weights (Wp = diag(ln_k_w) @ Wkv).
  - ln_k_b folds into c = ln_k_b @ Wkv. The k-part of c shifts every
    logit of a (head, query) row by the same constant, which softmax
    cancels exactly, so it is dropped. The v-part adds c_v to every
    attention output row (attention weights sum to 1), so it commutes
    past Wout: the kernel adds r = c_v @ Wout to the final output.

Device schedule (single pass over token chunks; small leading chunks
shrink the pipeline-fill bubble and dummy matmuls keep the PE HAM
clock-gate warm through it):
  - Per chunk: cast-load x tiles to bf16, LayerNorm on DVE, bounce the
    normalized tiles through DRAM, one large xbar DMA-transpose per
    d-tile into xnT, then: kT projection matmuls (e-major) -> sim
    matmuls for all 6 head pairs (simT[tok, query], K=64 row-pair
    packed) -> exp on ACT -> V projection matmuls -> attn@v chunk
    matmuls accumulated into per-head SBUF accumulators. Emitting sim
    before v/attn@v lets the ACT exp hide under PE work.
  - Softmax denominators come from a ones-column appended to v; max-
    subtraction is skipped (logits provably small for LN'd inputs).
  - Tail: per-head normalize (reciprocal partition-broadcast via a K=1
    ones-matmul) and the output projection (+ the c_v@Wout constant).
"""

import sys

sys.path.insert(0, "/opt/trn_rl_repo")

import numpy as np
import ml_dtypes

import concourse.bass as bass
import concourse.mybir as mybir
import concourse.tile as tile
from concourse import bacc
from concourse.bass_utils import run_bass_kernel_spmd

F32 = mybir.dt.float32
BF16 = mybir.dt.bfloat16
AX = mybir.AluOpType

B = 8
N_TOK = 4096
D_CTX = 1024
D_MODEL = 768
N_HEAD = 12
DH = 64
NQ = 256
INNER = 768
EPS = 1e-5
N_CORES = 8

TOK_TILES = N_TOK // 128  # 32
D_TILES = D_CTX // 128  # 8
E_TILES = INNER // 128  # 6


def emit_kernel(ctx, tc, out_d, x_d, wp_d, qt_d, wout_d, rrep_d, rep=0):
    nc = tc.nc
    xn_dram = nc.dram_tensor(f"xn_scratch{rep}", [N_TOK, D_CTX], BF16).ap()

    p_wp = ctx.enter_context(tc.tile_pool(name="wp", bufs=1))
    p_qt = ctx.enter_context(tc.tile_pool(name="qt", bufs=1))
    p_r = ctx.enter_context(tc.tile_pool(name="rr", bufs=1))
    p_x = ctx.enter_context(tc.tile_pool(name="x", bufs=3))
    p_xn = ctx.enter_context(tc.tile_pool(name="xn", bufs=2))
    p_big = ctx.enter_context(tc.tile_pool(name="big", bufs=2))
    p_attn = ctx.enter_context(tc.tile_pool(name="attn", bufs=3))
    p_kt = ctx.enter_context(tc.tile_pool(name="kt", bufs=E_TILES))
    p_v = ctx.enter_context(tc.tile_pool(name="v", bufs=TOK_TILES))
    p_acc = ctx.enter_context(tc.tile_pool(name="acc", bufs=N_HEAD))
    p_stat = ctx.enter_context(tc.tile_pool(name="stat", bufs=4))
    p_ot = ctx.enter_context(tc.tile_pool(name="ot", bufs=N_HEAD))
    p_fin = ctx.enter_context(tc.tile_pool(name="fin", bufs=1))
    p_rc = ctx.enter_context(tc.tile_pool(name="rc", bufs=2))
    ps_kv = ctx.enter_context(tc.tile_pool(name="pskv", bufs=2, space="PSUM"))
    ps_sim = ctx.enter_context(tc.tile_pool(name="pssim", bufs=2, space="PSUM"))
    ps_av = ctx.enter_context(tc.tile_pool(name="psav", bufs=2, space="PSUM"))

    # --- LN(x) preprocessing for one x-tile ------------------------------
    def prep_tile(i):
        xt = p_x.tile([128, D_CTX], BF16, tag="x", name=f"x{i}")
        nc.gpsimd.dma_start(out=xt[:], in_=x_d[i * 128 : (i + 1) * 128, :])

        st = p_stat.tile([128, 2, 6], F32, tag="st", name=f"st{i}")
        nc.vector.bn_stats(out=st[:, 0, :], in_=xt[:, 0:512])
        nc.vector.bn_stats(out=st[:, 1, :], in_=xt[:, 512:1024])
        mv = p_stat.tile([128, 2], F32, tag="mv", name=f"mv{i}")
        nc.vector.bn_aggr(out=mv[:], in_=st[:])
        rstd = p_stat.tile([128, 1], F32, tag="rstd", name=f"rstd{i}")
        nc.scalar.activation(
            out=rstd[:],
            in_=mv[:, 1:2],
            func=mybir.ActivationFunctionType.Sqrt,
            bias=eps_t[:],
            scale=1.0,
        )
        nc.vector.reciprocal(out=rstd[:], in_=rstd[:])
        negmr = p_stat.tile([128, 1], F32, tag="negmr", name=f"negmr{i}")
        nc.vector.scalar_tensor_tensor(
            out=negmr[:],
            in0=mv[:, 0:1],
            scalar=-1.0,
            in1=rstd[:],
            op0=AX.mult,
            op1=AX.mult,
        )
        xn = p_xn.tile([128, D_CTX], BF16, tag="xn", name=f"xn{i}")
        nc.vector.tensor_scalar(
            out=xn[:],
            in0=xt[:],
            scalar1=rstd[:, 0:1],
            scalar2=negmr[:, 0:1],
            op0=AX.mult,
            op1=AX.add,
        )
        nc.sync.dma_start(out=xn_dram[i * 128 : (i + 1) * 128, :], in_=xn[:])

    # --- get the x pipeline going before the big weight loads ------------
    eps_t = None  # placed below; prep_tile uses it via closure
    eps_t = tc.tile_pool(name="eps", bufs=1)
    p_eps = ctx.enter_context(eps_t)
    eps_t = p_eps.tile([128, 1], F32, tag="eps")
    nc.vector.memset(eps_t[:], EPS)
    for i in range(6):
        prep_tile(i)

    # --- constant loads (after the quarter-0 x pipeline is in flight).
    # Split the kv weights into k/v halves so the first kT matmuls only
    # wait on the k half.
    wp = p_wp.tile([128, D_TILES, 2 * INNER], BF16, tag="wp")
    wp_r = wp_d.rearrange("(t p) n -> p t n", p=128)
    nc.gpsimd.dma_start(out=wp[:, :, 0:INNER], in_=wp_r[:, :, 0:INNER])
    qt = p_qt.tile([128, E_TILES, NQ], BF16)
    nc.gpsimd.dma_start(out=qt[:], in_=qt_d.rearrange("(t p) n -> p t n", p=128))
    nc.gpsimd.dma_start(
        out=wp[:, :, INNER : 2 * INNER], in_=wp_r[:, :, INNER : 2 * INNER]
    )

    kt_tiles = []
    for e in range(E_TILES):
        kt_tiles.append(p_kt.tile([128, N_TOK], BF16, tag="kt", name=f"kt{e}"))
    v_tiles = []
    for j in range(TOK_TILES):
        v_tiles.append(p_v.tile([128, N_HEAD, DH + 1], BF16, tag="v", name=f"v{j}"))

    av_acc = []
    for h in range(N_HEAD):
        av_acc.append(p_acc.tile([DH + 1, NQ], F32, tag="acc", name=f"acc{h}"))
    ot_tiles = []
    rrep = p_r.tile([128, D_MODEL], F32)
    nc.gpsimd.dma_start(out=rrep[:], in_=rrep_d[:])
    ones_t = p_r.tile([128, DH], F32, tag="ones")
    nc.vector.memset(ones_t[:], 1.0)

    # PE warm-up: ~3.5us of dummy matmuls during the pipeline-fill window
    # so the HAM clock gate is at full rate when the real work arrives.
    warm = p_r.tile([128, 512], BF16, tag="warm")
    nc.vector.memset(warm[:], 1.0)
    wps = ps_sim.tile([128, 4, NQ], F32, tag="ps", name="warmps")
    for _ in range(18):
        nc.tensor.matmul(
            out=wps.rearrange("p a b -> p (a b)")[:, 0:512],
            lhsT=warm[:, 0:128],
            rhs=warm[:],
            start=True,
            stop=True,
        )

    # --- main pass: per quarter: LN -> xnT -> kT -> sim -> v -> attn@v ---
    # small leading quarters cut the pipeline-fill bubble before the
    # first matmul
    qsizes = [2, 2, 4, 8, 8, 8]
    qstarts = [sum(qsizes[:g]) for g in range(len(qsizes))]
    prepped = 6
    for q, (j0, nj) in enumerate(zip(qstarts, qsizes)):
        last_q = q == len(qsizes) - 1
        for _ in range(nj):
            if prepped < TOK_TILES:
                prep_tile(prepped)
                prepped += 1
        xnt = p_big.tile([128, D_TILES, nj * 128], BF16, tag="big",
                         name=f"xnt{q}")
        for d in range(D_TILES):
            nc.sync.dma_start(
                out=xnt[:, d, :],
                in_=xn_dram[j0 * 128 : (j0 + nj) * 128, d * 128 : (d + 1) * 128],
                transpose=True,
            )
        for e in range(E_TILES):
            for n2 in range(max(1, nj * 128 // 512)):
                nw = min(512, nj * 128)
                ps = ps_kv.tile([128, 512], F32, tag="ps", name=f"pkt{q}_{e}_{n2}")
                for d in range(D_TILES):
                    nc.tensor.matmul(
                        out=ps[:, 0:nw],
                        lhsT=wp[:, d, e * 128 : (e + 1) * 128],
                        rhs=xnt[:, d, n2 * 512 : n2 * 512 + nw],
                        start=(d == 0),
                        stop=(d == D_TILES - 1),
                    )
                c0 = j0 * 128 + n2 * 512
                nc.vector.tensor_copy(
                    out=kt_tiles[e][:, c0 : c0 + nw], in_=ps[:, 0:nw]
                )

        # sim + exp for all pairs over this quarter's token tiles. The
        # head-A (rows 0:64) and head-B (rows 64:128) matmuls of a group
        # are emitted adjacently so the PE runs the disjoint row-groups
        # concurrently (K=64 packing).
        attn_tiles = {}
        for p in range(E_TILES):
            for hh in range(2):
                attn_tiles[(p, hh)] = p_attn.tile(
                    [128, nj, NQ], BF16, tag="attn", name=f"at{q}_{p}_{hh}"
                )
            for g0 in range(0, nj, 4):
                ng = min(4, nj - g0)
                pss = {}
                for hh, base in ((0, 0), (1, 64)):
                    pss[hh] = ps_sim.tile([128, 4, NQ], F32, tag="ps",
                                          name=f"psim{q}_{p}_{hh}_{g0}")
                for jj in range(ng):
                    j = j0 + g0 + jj
                    for hh, base in ((0, 0), (1, 64)):
                        nc.tensor.matmul(
                            out=pss[hh][:, jj, :],
                            lhsT=kt_tiles[p][base : base + 64,
                                             j * 128 : (j + 1) * 128],
                            rhs=qt[base : base + 64, p, :],
                            start=True,
                            stop=True,
                        )
                for hh in range(2):
                    nc.scalar.activation(
                        out=attn_tiles[(p, hh)][:, g0 : g0 + ng, :],
                        in_=pss[hh][:, 0:ng, :],
                        func=mybir.ActivationFunctionType.Exp,
                    )

        # v projection for this quarter
        for jj in range(nj):
            j = j0 + jj
            vt = v_tiles[j]
            nc.vector.memset(vt[:, :, DH : DH + 1], 1.0)
            for h6 in range(2):
                ps = ps_kv.tile([128, 384], F32, tag="ps", name=f"pv{j}_{h6}")
                for d in range(D_TILES):
                    nc.tensor.matmul(
                        out=ps[:],
                        lhsT=xnt[:, d, jj * 128 : (jj + 1) * 128],
                        rhs=wp[:, d, INNER + h6 * 384 : INNER + (h6 + 1) * 384],
                        start=(d == 0),
                        stop=(d == D_TILES - 1),
                    )
                nc.vector.tensor_copy(
                    out=vt[:, h6 * 6 : (h6 + 1) * 6, 0:DH],
                    in_=ps.rearrange("p (h dh) -> p h dh", dh=DH),
                )

        # attn@v chunks for all pairs
        for p in range(E_TILES):
            for hh in range(2):
                h = 2 * p + hh
                attn = attn_tiles[(p, hh)]
                psa = ps_av.tile([DH + 1, NQ], F32, tag="ps", name=f"pav{q}_{h}")
                for jj in range(nj):
                    nc.tensor.matmul(
                        out=psa[:],
                        lhsT=v_tiles[j0 + jj][:, h, :],
                        rhs=attn[:, jj, :],
                        start=(jj == 0),
                        stop=(jj == nj - 1),
                    )
                if q == 0:
                    nc.vector.tensor_copy(out=av_acc[h][:], in_=psa[:])
                else:
                    nc.vector.tensor_tensor(
                        out=av_acc[h][:], in0=av_acc[h][:], in1=psa[:], op=AX.add
                    )
                if last_q:
                    # phase C inline: per-head softmax normalize as soon as
                    # the last chunk lands. The reciprocal is partition-
                    # broadcast with a K=1 ones-matmul into PSUM.
                    rc_sb = p_rc.tile([128, NQ], F32, tag="rcsb", name=f"rc{h}")
                    nc.vector.reciprocal(
                        out=rc_sb[DH : DH + 1, :], in_=av_acc[h][DH : DH + 1, :]
                    )
                    ps_rc = ps_sim.tile([DH, NQ], F32, tag="ps", name=f"psrc{h}")
                    nc.tensor.matmul(
                        out=ps_rc[:],
                        lhsT=ones_t[DH : DH + 1, 0:DH],
                        rhs=rc_sb[DH : DH + 1, :],
                        start=True,
                        stop=True,
                    )
                    ot = p_ot.tile([DH, NQ], BF16, tag="ot", name=f"ot{h}")
                    nc.vector.tensor_tensor(
                        out=ot[:],
                        in0=av_acc[h][0:DH, :],
                        in1=ps_rc[:],
                        op=AX.mult,
                    )
                    ot_tiles.append(ot)

    # wout reuses the wp slot (projections no longer need the kv weights)
    wout = p_wp.tile([DH, N_HEAD, D_MODEL], BF16, tag="wp")
    nc.gpsimd.dma_start(out=wout[:], in_=wout_d[:])

    # --- phase D: output projection --------------------------------------
    for q2 in range(NQ // 128):
        fin = p_fin.tile([128, D_MODEL], F32, tag="fin", name=f"fin{q2}")
        for n2 in range(2):
            psf = ps_kv.tile([128, 384], F32, tag="ps", name=f"pf{q2}_{n2}")
            for h in range(N_HEAD):
                nc.tensor.matmul(
                    out=psf[:],
                    lhsT=ot_tiles[h][:, q2 * 128 : (q2 + 1) * 128],
                    rhs=wout[:, h, n2 * 384 : (n2 + 1) * 384],
                    start=(h == 0),
                    stop=(h == N_HEAD - 1),
                )
            nc.vector.tensor_tensor(
                out=fin[:, n2 * 384 : (n2 + 1) * 384],
                in0=psf[:],
                in1=rrep[:, n2 * 384 : (n2 + 1) * 384],
                op=AX.add,
            )
        nc.sync.dma_start(out=out_d[q2 * 128 : (q2 + 1) * 128, :], in_=fin[:])


def build_nc(reps=1):
    nc = bacc.Bacc(
        "TRN2", target_bir_lowering=False, debug=False, num_devices=N_CORES
    )
    x_d = nc.dram_tensor("x", [N_TOK, D_CTX], F32, kind="ExternalInput").ap()
    wp_d = nc.dram_tensor("wp", [D_CTX, 2 * INNER], BF16, kind="ExternalInput").ap()
    qt_d = nc.dram_tensor("qt", [INNER, NQ], BF16, kind="ExternalInput").ap()
    wout_d = nc.dram_tensor(
        "wout", [DH, N_HEAD, D_MODEL], BF16, kind="ExternalInput"
    ).ap()
    rrep_d = nc.dram_tensor("rrep", [128, D_MODEL], F32, kind="ExternalInput").ap()
    out_d = nc.dram_tensor("out", [NQ, D_MODEL], F32, kind="ExternalOutput").ap()
    from contextlib import ExitStack

    with tile.TileContext(nc) as tc:
        for rep in range(reps):
            with ExitStack() as ctx:
                emit_kernel(ctx, tc, out_d, x_d, wp_d, qt_d, wout_d, rrep_d, rep=rep)
    nc.compile()
    return nc


def host_prep(query, ln_q_w, ln_q_b, ln_k_w, ln_k_b, Wq, Wkv, Wout):
    """Batch-independent fp32 preprocessing. Returns per-core input dict
    (minus x)."""
    query = np.asarray(query, np.float32)
    mu = query.mean(-1, keepdims=True)
    var = ((query - mu) ** 2).mean(-1, keepdims=True)
    qn = (query - mu) / np.sqrt(var + EPS) * ln_q_w + ln_q_b
    qmat = (qn @ np.asarray(Wq, np.float32)) * (DH**-0.5)  # [NQ, INNER]
    qT = np.ascontiguousarray(qmat.T).astype(ml_dtypes.bfloat16)

    Wkv = np.asarray(Wkv, np.float32)
    Wp = (np.asarray(ln_k_w, np.float32)[:, None] * Wkv).astype(ml_dtypes.bfloat16)
    c = np.asarray(ln_k_b, np.float32) @ Wkv  # [2*INNER]
    c_v = c[INNER:]
    Wout = np.asarray(Wout, np.float32)
    r = c_v @ Wout  # [D_MODEL]
    rrep = np.ascontiguousarray(np.broadcast_to(r, (128, D_MODEL))).astype(np.float32)
    wout_arr = np.ascontiguousarray(
        Wout.reshape(N_HEAD, DH, D_MODEL).transpose(1, 0, 2)
    ).astype(ml_dtypes.bfloat16)
    return {"wp": Wp, "qt": qT, "wout": wout_arr, "rrep": rrep}


_NC_CACHE = {}


def get_nc():
    if "nc" not in _NC_CACHE:
        _NC_CACHE["nc"] = build_nc()
    return _NC_CACHE["nc"]


def kernel(x, query, ln_q_w, ln_q_b, ln_k_w, ln_k_b, Wq, Wkv, Wout):
    x = np.asarray(x, np.float32)
    shared = host_prep(query, ln_q_w, ln_q_b, ln_k_w, ln_k_b, Wq, Wkv, Wout)
    in_maps = [
        {"x": np.ascontiguousarray(x[b]), **shared} for b in range(B)
    ]
    nc = get_nc()
    res = run_bass_kernel_spmd(nc, in_maps, list(range(N_CORES)))
    return np.stack([res.results[b]["out"] for b in range(B)], axis=0)


# revision 36
# speedup vs baseline: 1.0165x; 1.0165x over previous
"""AttentionalPooler Trainium2 kernel.

Full inputs -> full output; batch (8) is data-parallel across the 8
NeuronCores. Per core: LayerNorm(x_b), kv = LN(x_b) @ Wkv, 12-head
cross-attention from 256 pre-computed queries, output projection.

Host-side preprocessing (exact fp32 algebra, batch-independent):
  - q path (LN(query) @ Wq * dh^-0.5, transposed) is computed on host.
  - ln_k_w is folded into the kv weights (Wp = diag(ln_k_w) @ Wkv).
  - ln_k_b folds into c = ln_k_b @ Wkv. The k-part of c shifts every
    logit of a (head, query) row by the same constant, which softmax
    cancels exactly, so it is dropped. The v-part adds c_v to every
    attention output row (attention weights sum to 1), so it commutes
    past Wout: the kernel adds r = c_v @ Wout to the final output.

Device schedule (single pass over token chunks; small leading chunks
shrink the pipeline-fill bubble and dummy matmuls keep the PE HAM
clock-gate warm through it):
  - Per chunk: cast-load x tiles to bf16, LayerNorm on DVE, bounce the
    normalized tiles through DRAM, one large xbar DMA-transpose per
    d-tile into xnT, then: kT projection matmuls (e-major) -> sim
    matmuls for all 6 head pairs (simT[tok, query], K=64 row-pair
    packed) -> exp on ACT -> V projection matmuls -> attn@v chunk
    matmuls accumulated into per-head SBUF accumulators. Emitting sim
    before v/attn@v lets the ACT exp hide under PE work.
  - Softmax denominators come from a ones-column appended to v; max-
    subtraction is skipped (logits provably small for LN'd inputs).
  - Tail: per-head normalize (reciprocal partition-broadcast via a K=1
    ones-matmul) and the output projection (+ the c_v@Wout constant).
"""

import sys

sys.path.insert(0, "/opt/trn_rl_repo")

import numpy as np
import ml_dtypes

import concourse.bass as bass
import concourse.mybir as mybir
import concourse.tile as tile
from concourse import bacc
from concourse.bass_utils import run_bass_kernel_spmd

F32 = mybir.dt.float32
BF16 = mybir.dt.bfloat16
AX = mybir.AluOpType

B = 8
N_TOK = 4096
D_CTX = 1024
D_MODEL = 768
N_HEAD = 12
DH = 64
NQ = 256
INNER = 768
EPS = 1e-5
N_CORES = 8

TOK_TILES = N_TOK // 128  # 32
D_TILES = D_CTX // 128  # 8
E_TILES = INNER // 128  # 6


def emit_kernel(ctx, tc, out_d, x_d, wp_d, qt_d, wout_d, rrep_d, rep=0):
    nc = tc.nc
    xn_dram = nc.dram_tensor(f"xn_scratch{rep}", [N_TOK, D_CTX], BF16).ap()

    p_wp = ctx.enter_context(tc.tile_pool(name="wp", bufs=1))
    p_qt = ctx.enter_context(tc.tile_pool(name="qt", bufs=1))
    p_r = ctx.enter_context(tc.tile_pool(name="rr", bufs=1))
    p_x = ctx.enter_context(tc.tile_pool(name="x", bufs=3))
    p_xn = ctx.enter_context(tc.tile_pool(name="xn", bufs=2))
    p_big = ctx.enter_context(tc.tile_pool(name="big", bufs=2))
    p_attn = ctx.enter_context(tc.tile_pool(name="attn", bufs=3))
    p_kt = ctx.enter_context(tc.tile_pool(name="kt", bufs=E_TILES))
    p_v = ctx.enter_context(tc.tile_pool(name="v", bufs=TOK_TILES))
    p_acc = ctx.enter_context(tc.tile_pool(name="acc", bufs=N_HEAD))
    p_stat = ctx.enter_context(tc.tile_pool(name="stat", bufs=4))
    p_ot = ctx.enter_context(tc.tile_pool(name="ot", bufs=N_HEAD))
    p_fin = ctx.enter_context(tc.tile_pool(name="fin", bufs=1))
    p_rc = ctx.enter_context(tc.tile_pool(name="rc", bufs=2))
    ps_kv = ctx.enter_context(tc.tile_pool(name="pskv", bufs=2, space="PSUM"))
    ps_sim = ctx.enter_context(tc.tile_pool(name="pssim", bufs=2, space="PSUM"))
    ps_av = ctx.enter_context(tc.tile_pool(name="psav", bufs=2, space="PSUM"))

    # --- LN(x) preprocessing for one x-tile ------------------------------
    def prep_tile(i):
        xt = p_x.tile([128, D_CTX], BF16, tag="x", name=f"x{i}")
        nc.gpsimd.dma_start(out=xt[:], in_=x_d[i * 128 : (i + 1) * 128, :])

        st = p_stat.tile([128, 2, 6], F32, tag="st", name=f"st{i}")
        nc.vector.bn_stats(out=st[:, 0, :], in_=xt[:, 0:512])
        nc.vector.bn_stats(out=st[:, 1, :], in_=xt[:, 512:1024])
        mv = p_stat.tile([128, 2], F32, tag="mv", name=f"mv{i}")
        nc.vector.bn_aggr(out=mv[:], in_=st[:])
        rstd = p_stat.tile([128, 1], F32, tag="rstd", name=f"rstd{i}")
        nc.scalar.activation(
            out=rstd[:],
            in_=mv[:, 1:2],
            func=mybir.ActivationFunctionType.Sqrt,
            bias=eps_t[:],
            scale=1.0,
        )
        nc.vector.reciprocal(out=rstd[:], in_=rstd[:])
        negmr = p_stat.tile([128, 1], F32, tag="negmr", name=f"negmr{i}")
        nc.vector.scalar_tensor_tensor(
            out=negmr[:],
            in0=mv[:, 0:1],
            scalar=-1.0,
            in1=rstd[:],
            op0=AX.mult,
            op1=AX.mult,
        )
        xn = p_xn.tile([128, D_CTX], BF16, tag="xn", name=f"xn{i}")
        nc.vector.tensor_scalar(
            out=xn[:],
            in0=xt[:],
            scalar1=rstd[:, 0:1],
            scalar2=negmr[:, 0:1],
            op0=AX.mult,
            op1=AX.add,
        )
        nc.sync.dma_start(out=xn_dram[i * 128 : (i + 1) * 128, :], in_=xn[:])

    # --- get the x pipeline going before the big weight loads ------------
    eps_t = None  # placed below; prep_tile uses it via closure
    eps_t = tc.tile_pool(name="eps", bufs=1)
    p_eps = ctx.enter_context(eps_t)
    eps_t = p_eps.tile([128, 1], F32, tag="eps")
    nc.vector.memset(eps_t[:], EPS)
    for i in range(6):
        prep_tile(i)

    # --- constant loads (after the quarter-0 x pipeline is in flight).
    # Split the kv weights into k/v halves so the first kT matmuls only
    # wait on the k half.
    wp = p_wp.tile([128, D_TILES, 2 * INNER], BF16, tag="wp")
    wp_r = wp_d.rearrange("(t p) n -> p t n", p=128)
    nc.gpsimd.dma_start(out=wp[:, :, 0:INNER], in_=wp_r[:, :, 0:INNER])
    qt = p_qt.tile([128, E_TILES, NQ], BF16)
    nc.gpsimd.dma_start(out=qt[:], in_=qt_d.rearrange("(t p) n -> p t n", p=128))
    nc.gpsimd.dma_start(
        out=wp[:, :, INNER : 2 * INNER], in_=wp_r[:, :, INNER : 2 * INNER]
    )

    kt_tiles = []
    for e in range(E_TILES):
        kt_tiles.append(p_kt.tile([128, N_TOK], BF16, tag="kt", name=f"kt{e}"))
    v_tiles = []
    for j in range(TOK_TILES):
        v_tiles.append(p_v.tile([128, N_HEAD, DH + 1], BF16, tag="v", name=f"v{j}"))

    av_acc = []
    for h in range(N_HEAD):
        av_acc.append(p_acc.tile([DH + 1, NQ], F32, tag="acc", name=f"acc{h}"))
    ot_tiles = []
    rrep = p_r.tile([128, D_MODEL], F32)
    nc.gpsimd.dma_start(out=rrep[:], in_=rrep_d[:])
    ones_t = p_r.tile([128, DH], F32, tag="ones")
    nc.vector.memset(ones_t[:], 1.0)

    # PE warm-up: ~3.5us of dummy matmuls during the pipeline-fill window
    # so the HAM clock gate is at full rate when the real work arrives.
    warm = p_r.tile([128, 512], BF16, tag="warm")
    nc.vector.memset(warm[:], 1.0)
    wps = ps_sim.tile([128, 4, NQ], F32, tag="ps", name="warmps")
    for _ in range(18):
        nc.tensor.matmul(
            out=wps.rearrange("p a b -> p (a b)")[:, 0:512],
            lhsT=warm[:, 0:128],
            rhs=warm[:],
            start=True,
            stop=True,
        )

    # --- main pass: per quarter: LN -> xnT -> kT -> sim -> v -> attn@v ---
    # small leading quarters cut the pipeline-fill bubble before the
    # first matmul
    qsizes = [2, 2, 4, 8, 8, 8]
    qstarts = [sum(qsizes[:g]) for g in range(len(qsizes))]
    prepped = 6
    for q, (j0, nj) in enumerate(zip(qstarts, qsizes)):
        last_q = q == len(qsizes) - 1
        for _ in range(nj):
            if prepped < TOK_TILES:
                prep_tile(prepped)
                prepped += 1
        xnt = p_big.tile([128, D_TILES, nj * 128], BF16, tag="big",
                         name=f"xnt{q}")
        for d in range(D_TILES):
            nc.sync.dma_start(
                out=xnt[:, d, :],
                in_=xn_dram[j0 * 128 : (j0 + nj) * 128, d * 128 : (d + 1) * 128],
                transpose=True,
            )
        for e in range(E_TILES):
            for n2 in range(max(1, nj * 128 // 512)):
                nw = min(512, nj * 128)
                ps = ps_kv.tile([128, 512], F32, tag="ps", name=f"pkt{q}_{e}_{n2}")
                for d in range(D_TILES):
                    nc.tensor.matmul(
                        out=ps[:, 0:nw],
                        lhsT=wp[:, d, e * 128 : (e + 1) * 128],
                        rhs=xnt[:, d, n2 * 512 : n2 * 512 + nw],
                        start=(d == 0),
                        stop=(d == D_TILES - 1),
                    )
                c0 = j0 * 128 + n2 * 512
                nc.vector.tensor_copy(
                    out=kt_tiles[e][:, c0 : c0 + nw], in_=ps[:, 0:nw]
                )

        # sim + exp for all pairs over this quarter's token tiles
        attn_tiles = {}
        for p in range(E_TILES):
            for hh, base in ((0, 0), (1, 64)):
                attn = p_attn.tile([128, nj, NQ], BF16, tag="attn",
                                   name=f"at{q}_{p}_{hh}")
                attn_tiles[(p, hh)] = attn
                for g0 in range(0, nj, 4):
                    ng = min(4, nj - g0)
                    ps = ps_sim.tile([128, 4, NQ], F32, tag="ps",
                                     name=f"psim{q}_{p}_{hh}_{g0}")
                    for jj in range(ng):
                        j = j0 + g0 + jj
                        nc.tensor.matmul(
                            out=ps[:, jj, :],
                            lhsT=kt_tiles[p][base : base + 64,
                                             j * 128 : (j + 1) * 128],
                            rhs=qt[base : base + 64, p, :],
                            start=True,
                            stop=True,
                        )
                    nc.scalar.activation(
                        out=attn[:, g0 : g0 + ng, :],
                        in_=ps[:, 0:ng, :],
                        func=mybir.ActivationFunctionType.Exp,
                    )

        # v projection for this quarter
        for jj in range(nj):
            j = j0 + jj
            vt = v_tiles[j]
            nc.vector.memset(vt[:, :, DH : DH + 1], 1.0)
            for h6 in range(2):
                ps = ps_kv.tile([128, 384], F32, tag="ps", name=f"pv{j}_{h6}")
                for d in range(D_TILES):
                    nc.tensor.matmul(
                        out=ps[:],
                        lhsT=xnt[:, d, jj * 128 : (jj + 1) * 128],
                        rhs=wp[:, d, INNER + h6 * 384 : INNER + (h6 + 1) * 384],
                        start=(d == 0),
                        stop=(d == D_TILES - 1),
                    )
                nc.vector.tensor_copy(
                    out=vt[:, h6 * 6 : (h6 + 1) * 6, 0:DH],
                    in_=ps.rearrange("p (h dh) -> p h dh", dh=DH),
                )

        # attn@v chunks for all pairs
        for p in range(E_TILES):
            for hh in range(2):
                h = 2 * p + hh
                attn = attn_tiles[(p, hh)]
                psa = ps_av.tile([DH + 1, NQ], F32, tag="ps", name=f"pav{q}_{h}")
                for jj in range(nj):
                    nc.tensor.matmul(
                        out=psa[:],
                        lhsT=v_tiles[j0 + jj][:, h, :],
                        rhs=attn[:, jj, :],
                        start=(jj == 0),
                        stop=(jj == nj - 1),
                    )
                if q == 0:
                    nc.vector.tensor_copy(out=av_acc[h][:], in_=psa[:])
                else:
                    nc.vector.tensor_tensor(
                        out=av_acc[h][:], in0=av_acc[h][:], in1=psa[:], op=AX.add
                    )
                if last_q:
                    # phase C inline: per-head softmax normalize as soon as
                    # the last chunk lands. The reciprocal is partition-
                    # broadcast with a K=1 ones-matmul into PSUM.
                    rc_sb = p_rc.tile([128, NQ], F32, tag="rcsb", name=f"rc{h}")
                    nc.vector.reciprocal(
                        out=rc_sb[DH : DH + 1, :], in_=av_acc[h][DH : DH + 1, :]
                    )
                    ps_rc = ps_sim.tile([DH, NQ], F32, tag="ps", name=f"psrc{h}")
                    nc.tensor.matmul(
                        out=ps_rc[:],
                        lhsT=ones_t[DH : DH + 1, 0:DH],
                        rhs=rc_sb[DH : DH + 1, :],
                        start=True,
                        stop=True,
                    )
                    ot = p_ot.tile([DH, NQ], BF16, tag="ot", name=f"ot{h}")
                    nc.vector.tensor_tensor(
                        out=ot[:],
                        in0=av_acc[h][0:DH, :],
                        in1=ps_rc[:],
                        op=AX.mult,
                    )
                    ot_tiles.append(ot)

    # wout reuses the wp slot (projections no longer need the kv weights)
    wout = p_wp.tile([DH, N_HEAD, D_MODEL], BF16, tag="wp")
    nc.gpsimd.dma_start(out=wout[:], in_=wout_d[:])

    # --- phase D: output projection --------------------------------------
    for q2 in range(NQ // 128):
        fin = p_fin.tile([128, D_MODEL], F32, tag="fin", name=f"fin{q2}")
        for n2 in range(2):
            psf = ps_kv.tile([128, 384], F32, tag="ps", name=f"pf{q2}_{n2}")
            for h in range(N_HEAD):
                nc.tensor.matmul(
                    out=psf[:],
                    lhsT=ot_tiles[h][:, q2 * 128 : (q2 + 1) * 128],
                    rhs=wout[:, h, n2 * 384 : (n2 + 1) * 384],
                    start=(h == 0),
                    stop=(h == N_HEAD - 1),
                )
            nc.vector.tensor_tensor(
                out=fin[:, n2 * 384 : (n2 + 1) * 384],
                in0=psf[:],
                in1=rrep[:, n2 * 384 : (n2 + 1) * 384],
                op=AX.add,
            )
        nc.sync.dma_start(out=out_d[q2 * 128 : (q2 + 1) * 128, :], in_=fin[:])


def build_nc(reps=1):
    nc = bacc.Bacc(
        "TRN2", target_bir_lowering=False, debug=False, num_devices=N_CORES
    )
    x_d = nc.dram_tensor("x", [N_TOK, D_CTX], F32, kind="ExternalInput").ap()
    wp_d = nc.dram_tensor("wp", [D_CTX, 2 * INNER], BF16, kind="ExternalInput").ap()
    qt_d = nc.dram_tensor("qt", [INNER, NQ], BF16, kind="ExternalInput").ap()
    wout_d = nc.dram_tensor(
        "wout", [DH, N_HEAD, D_MODEL], BF16, kind="ExternalInput"
    ).ap()
    rrep_d = nc.dram_tensor("rrep", [128, D_MODEL], F32, kind="ExternalInput").ap()
    out_d = nc.dram_tensor("out", [NQ, D_MODEL], F32, kind="ExternalOutput").ap()
    from contextlib import ExitStack

    with tile.TileContext(nc) as tc:
        for rep in range(reps):
            with ExitStack() as ctx:
                emit_kernel(ctx, tc, out_d, x_d, wp_d, qt_d, wout_d, rrep_d, rep=rep)
    nc.compile()
    return nc


def host_prep(query, ln_q_w, ln_q_b, ln_k_w, ln_k_b, Wq, Wkv, Wout):
    """Batch-independent fp32 preprocessing. Returns per-core input dict
    (minus x)."""
    query = np.asarray(query, np.float32)
    mu = query.mean(-1, keepdims=True)
    var = ((query - mu) ** 2).mean(-1, keepdims=True)
    qn = (query - mu) / np.sqrt(var + EPS) * ln_q_w + ln_q_b
    qmat = (qn @ np.asarray(Wq, np.float32)) * (DH**-0.5)  # [NQ, INNER]
    qT = np.ascontiguousarray(qmat.T).astype(ml_dtypes.bfloat16)

    Wkv = np.asarray(Wkv, np.float32)
    Wp = (np.asarray(ln_k_w, np.float32)[:, None] * Wkv).astype(ml_dtypes.bfloat16)
    c = np.asarray(ln_k_b, np.float32) @ Wkv  # [2*INNER]
    c_v = c[INNER:]
    Wout = np.asarray(Wout, np.float32)
    r = c_v @ Wout  # [D_MODEL]
    rrep = np.ascontiguousarray(np.broadcast_to(r, (128, D_MODEL))).astype(np.float32)
    wout_arr = np.ascontiguousarray(
        Wout.reshape(N_HEAD, DH, D_MODEL).transpose(1, 0, 2)
    ).astype(ml_dtypes.bfloat16)
    return {"wp": Wp, "qt": qT, "wout": wout_arr, "rrep": rrep}


_NC_CACHE = {}


def get_nc():
    if "nc" not in _NC_CACHE:
        _NC_CACHE["nc"] = build_nc()
    return _NC_CACHE["nc"]


def kernel(x, query, ln_q_w, ln_q_b, ln_k_w, ln_k_b, Wq, Wkv, Wout):
    x = np.asarray(x, np.float32)
    shared = host_prep(query, ln_q_w, ln_q_b, ln_k_w, ln_k_b, Wq, Wkv, Wout)
    in_maps = [
        {"x": np.ascontiguousarray(x[b]), **shared} for b in range(B)
    ]
    nc = get_nc()
    res = run_bass_kernel_spmd(nc, in_maps, list(range(N_CORES)))
    return np.stack([res.results[b]["out"] for b in range(B)], axis=0)


# revision 37
# speedup vs baseline: 1.0299x; 1.0131x over previous
"""AttentionalPooler Trainium2 kernel.

Full inputs -> full output; batch (8) is data-parallel across the 8
NeuronCores. Per core: LayerNorm(x_b), kv = LN(x_b) @ Wkv, 12-head
cross-attention from 256 pre-computed queries, output projection.

Host-side preprocessing (exact fp32 algebra, batch-independent):
  - q path (LN(query) @ Wq * dh^-0.5, transposed) is computed on host.
  - ln_k_w is folded into the kv weights (Wp = diag(ln_k_w) @ Wkv).
  - ln_k_b folds into c = ln_k_b @ Wkv. The k-part of c shifts every
    logit of a (head, query) row by the same constant, which softmax
    cancels exactly, so it is dropped. The v-part adds c_v to every
    attention output row (attention weights sum to 1), so it commutes
    past Wout: the kernel adds r = c_v @ Wout to the final output.

Device schedule (single pass over token chunks; small leading chunks
shrink the pipeline-fill bubble and dummy matmuls keep the PE HAM
clock-gate warm through it):
  - Per chunk: cast-load x tiles to bf16, LayerNorm on DVE, bounce the
    normalized tiles through DRAM, one large xbar DMA-transpose per
    d-tile into xnT, then: kT projection matmuls (e-major) -> sim
    matmuls for all 6 head pairs (simT[tok, query], K=64 row-pair
    packed) -> exp on ACT -> V projection matmuls -> attn@v chunk
    matmuls accumulated into per-head SBUF accumulators. Emitting sim
    before v/attn@v lets the ACT exp hide under PE work.
  - Softmax denominators come from a ones-column appended to v; max-
    subtraction is skipped (logits provably small for LN'd inputs).
  - Tail: per-head normalize (reciprocal partition-broadcast via a K=1
    ones-matmul) and the output projection (+ the c_v@Wout constant).
"""

import sys

sys.path.insert(0, "/opt/trn_rl_repo")

import numpy as np
import ml_dtypes

import concourse.bass as bass
import concourse.mybir as mybir
import concourse.tile as tile
from concourse import bacc
from concourse.bass_utils import run_bass_kernel_spmd

F32 = mybir.dt.float32
BF16 = mybir.dt.bfloat16
AX = mybir.AluOpType

B = 8
N_TOK = 4096
D_CTX = 1024
D_MODEL = 768
N_HEAD = 12
DH = 64
NQ = 256
INNER = 768
EPS = 1e-5
N_CORES = 8

TOK_TILES = N_TOK // 128  # 32
D_TILES = D_CTX // 128  # 8
E_TILES = INNER // 128  # 6


def emit_kernel(ctx, tc, out_d, x_d, wp_d, qt_d, wout_d, rrep_d, rep=0):
    nc = tc.nc
    xn_dram = nc.dram_tensor(f"xn_scratch{rep}", [N_TOK, D_CTX], BF16).ap()

    p_wp = ctx.enter_context(tc.tile_pool(name="wp", bufs=1))
    p_qt = ctx.enter_context(tc.tile_pool(name="qt", bufs=1))
    p_r = ctx.enter_context(tc.tile_pool(name="rr", bufs=1))
    p_x = ctx.enter_context(tc.tile_pool(name="x", bufs=3))
    p_xn = ctx.enter_context(tc.tile_pool(name="xn", bufs=2))
    p_big = ctx.enter_context(tc.tile_pool(name="big", bufs=2))
    p_attn = ctx.enter_context(tc.tile_pool(name="attn", bufs=3))
    p_kt = ctx.enter_context(tc.tile_pool(name="kt", bufs=E_TILES))
    p_v = ctx.enter_context(tc.tile_pool(name="v", bufs=TOK_TILES))
    p_acc = ctx.enter_context(tc.tile_pool(name="acc", bufs=N_HEAD))
    p_stat = ctx.enter_context(tc.tile_pool(name="stat", bufs=4))
    p_ot = ctx.enter_context(tc.tile_pool(name="ot", bufs=N_HEAD))
    p_fin = ctx.enter_context(tc.tile_pool(name="fin", bufs=1))
    p_rc = ctx.enter_context(tc.tile_pool(name="rc", bufs=2))
    ps_kv = ctx.enter_context(tc.tile_pool(name="pskv", bufs=2, space="PSUM"))
    ps_sim = ctx.enter_context(tc.tile_pool(name="pssim", bufs=2, space="PSUM"))
    ps_av = ctx.enter_context(tc.tile_pool(name="psav", bufs=2, space="PSUM"))

    # --- LN(x) preprocessing for one x-tile ------------------------------
    def prep_tile(i):
        xt = p_x.tile([128, D_CTX], BF16, tag="x", name=f"x{i}")
        nc.gpsimd.dma_start(out=xt[:], in_=x_d[i * 128 : (i + 1) * 128, :])

        st = p_stat.tile([128, 2, 6], F32, tag="st", name=f"st{i}")
        nc.vector.bn_stats(out=st[:, 0, :], in_=xt[:, 0:512])
        nc.vector.bn_stats(out=st[:, 1, :], in_=xt[:, 512:1024])
        mv = p_stat.tile([128, 2], F32, tag="mv", name=f"mv{i}")
        nc.vector.bn_aggr(out=mv[:], in_=st[:])
        rstd = p_stat.tile([128, 1], F32, tag="rstd", name=f"rstd{i}")
        nc.scalar.activation(
            out=rstd[:],
            in_=mv[:, 1:2],
            func=mybir.ActivationFunctionType.Sqrt,
            bias=eps_t[:],
            scale=1.0,
        )
        nc.vector.reciprocal(out=rstd[:], in_=rstd[:])
        negmr = p_stat.tile([128, 1], F32, tag="negmr", name=f"negmr{i}")
        nc.vector.scalar_tensor_tensor(
            out=negmr[:],
            in0=mv[:, 0:1],
            scalar=-1.0,
            in1=rstd[:],
            op0=AX.mult,
            op1=AX.mult,
        )
        xn = p_xn.tile([128, D_CTX], BF16, tag="xn", name=f"xn{i}")
        nc.vector.tensor_scalar(
            out=xn[:],
            in0=xt[:],
            scalar1=rstd[:, 0:1],
            scalar2=negmr[:, 0:1],
            op0=AX.mult,
            op1=AX.add,
        )
        nc.sync.dma_start(out=xn_dram[i * 128 : (i + 1) * 128, :], in_=xn[:])

    # --- get the x pipeline going before the big weight loads ------------
    p_eps = ctx.enter_context(tc.tile_pool(name="eps", bufs=1))
    eps_t = p_eps.tile([128, 1], F32, tag="eps")
    nc.vector.memset(eps_t[:], EPS)
    for i in range(6):
        prep_tile(i)

    # --- constant loads (after the quarter-0 x pipeline is in flight).
    # Split the kv weights into k/v halves so the first kT matmuls only
    # wait on the k half.
    wp = p_wp.tile([128, D_TILES, 2 * INNER], BF16, tag="wp")
    wp_r = wp_d.rearrange("(t p) n -> p t n", p=128)
    nc.gpsimd.dma_start(out=wp[:, :, 0:INNER], in_=wp_r[:, :, 0:INNER])
    qt = p_qt.tile([128, E_TILES, NQ], BF16)
    nc.gpsimd.dma_start(out=qt[:], in_=qt_d.rearrange("(t p) n -> p t n", p=128))
    nc.gpsimd.dma_start(
        out=wp[:, :, INNER : 2 * INNER], in_=wp_r[:, :, INNER : 2 * INNER]
    )

    kt_tiles = []
    for e in range(E_TILES):
        kt_tiles.append(p_kt.tile([128, N_TOK], BF16, tag="kt", name=f"kt{e}"))
    v_tiles = []
    for j in range(TOK_TILES):
        v_tiles.append(p_v.tile([128, N_HEAD, DH + 1], BF16, tag="v", name=f"v{j}"))

    av_acc = []
    for h in range(N_HEAD):
        av_acc.append(p_acc.tile([DH + 1, NQ], F32, tag="acc", name=f"acc{h}"))
    ot_tiles = []
    rrep = p_r.tile([128, D_MODEL], F32)
    nc.gpsimd.dma_start(out=rrep[:], in_=rrep_d[:])
    ones_t = p_r.tile([128, DH], F32, tag="ones")
    nc.vector.memset(ones_t[:], 1.0)

    # PE warm-up: ~3.5us of dummy matmuls during the pipeline-fill window
    # so the HAM clock gate is at full rate when the real work arrives.
    warm = p_r.tile([128, 512], BF16, tag="warm")
    nc.vector.memset(warm[:], 1.0)
    wps = ps_sim.tile([128, 4, NQ], F32, tag="ps", name="warmps")
    for _ in range(18):
        nc.tensor.matmul(
            out=wps.rearrange("p a b -> p (a b)")[:, 0:512],
            lhsT=warm[:, 0:128],
            rhs=warm[:],
            start=True,
            stop=True,
        )

    # --- main pass: per quarter: LN -> xnT -> kT -> sim -> v -> attn@v ---
    # small leading quarters cut the pipeline-fill bubble before the
    # first matmul
    qsizes = [2, 2, 4, 8, 8, 8]
    qstarts = [sum(qsizes[:g]) for g in range(len(qsizes))]
    prepped = 6
    for q, (j0, nj) in enumerate(zip(qstarts, qsizes)):
        last_q = q == len(qsizes) - 1
        for _ in range(nj):
            if prepped < TOK_TILES:
                prep_tile(prepped)
                prepped += 1
        xnt = p_big.tile([128, D_TILES, nj * 128], BF16, tag="big",
                         name=f"xnt{q}")
        for d in range(D_TILES):
            nc.sync.dma_start(
                out=xnt[:, d, :],
                in_=xn_dram[j0 * 128 : (j0 + nj) * 128, d * 128 : (d + 1) * 128],
                transpose=True,
            )
        for e in range(E_TILES):
            for n2 in range(max(1, nj * 128 // 512)):
                nw = min(512, nj * 128)
                ps = ps_kv.tile([128, 512], F32, tag="ps", name=f"pkt{q}_{e}_{n2}")
                for d in range(D_TILES):
                    nc.tensor.matmul(
                        out=ps[:, 0:nw],
                        lhsT=wp[:, d, e * 128 : (e + 1) * 128],
                        rhs=xnt[:, d, n2 * 512 : n2 * 512 + nw],
                        start=(d == 0),
                        stop=(d == D_TILES - 1),
                    )
                c0 = j0 * 128 + n2 * 512
                nc.vector.tensor_copy(
                    out=kt_tiles[e][:, c0 : c0 + nw], in_=ps[:, 0:nw]
                )

        # sim + exp for all pairs over this quarter's token tiles
        attn_tiles = {}
        for p in range(E_TILES):
            for hh, base in ((0, 0), (1, 64)):
                attn = p_attn.tile([128, nj, NQ], BF16, tag="attn",
                                   name=f"at{q}_{p}_{hh}")
                attn_tiles[(p, hh)] = attn
                for g0 in range(0, nj, 4):
                    ng = min(4, nj - g0)
                    ps = ps_sim.tile([128, 4, NQ], F32, tag="ps",
                                     name=f"psim{q}_{p}_{hh}_{g0}")
                    for jj in range(ng):
                        j = j0 + g0 + jj
                        nc.tensor.matmul(
                            out=ps[:, jj, :],
                            lhsT=kt_tiles[p][base : base + 64,
                                             j * 128 : (j + 1) * 128],
                            rhs=qt[base : base + 64, p, :],
                            start=True,
                            stop=True,
                        )
                    nc.scalar.activation(
                        out=attn[:, g0 : g0 + ng, :],
                        in_=ps[:, 0:ng, :],
                        func=mybir.ActivationFunctionType.Exp,
                    )

        # v projection for this quarter
        for jj in range(nj):
            j = j0 + jj
            vt = v_tiles[j]
            nc.vector.memset(vt[:, :, DH : DH + 1], 1.0)
            for h6 in range(2):
                ps = ps_kv.tile([128, 384], F32, tag="ps", name=f"pv{j}_{h6}")
                for d in range(D_TILES):
                    nc.tensor.matmul(
                        out=ps[:],
                        lhsT=xnt[:, d, jj * 128 : (jj + 1) * 128],
                        rhs=wp[:, d, INNER + h6 * 384 : INNER + (h6 + 1) * 384],
                        start=(d == 0),
                        stop=(d == D_TILES - 1),
                    )
                nc.vector.tensor_copy(
                    out=vt[:, h6 * 6 : (h6 + 1) * 6, 0:DH],
                    in_=ps.rearrange("p (h dh) -> p h dh", dh=DH),
                )

        # attn@v chunks for all pairs
        for p in range(E_TILES):
            for hh in range(2):
                h = 2 * p + hh
                attn = attn_tiles[(p, hh)]
                psa = ps_av.tile([DH + 1, NQ], F32, tag="ps", name=f"pav{q}_{h}")
                for jj in range(nj):
                    nc.tensor.matmul(
                        out=psa[:],
                        lhsT=v_tiles[j0 + jj][:, h, :],
                        rhs=attn[:, jj, :],
                        start=(jj == 0),
                        stop=(jj == nj - 1),
                    )
                if q == 0:
                    nc.vector.tensor_copy(out=av_acc[h][:], in_=psa[:])
                else:
                    nc.vector.tensor_tensor(
                        out=av_acc[h][:], in0=av_acc[h][:], in1=psa[:], op=AX.add
                    )
                if last_q:
                    # phase C inline: per-head softmax normalize as soon as
                    # the last chunk lands. The reciprocal is partition-
                    # broadcast with a K=1 ones-matmul into PSUM.
                    rc_sb = p_rc.tile([128, NQ], F32, tag="rcsb", name=f"rc{h}")
                    nc.vector.reciprocal(
                        out=rc_sb[DH : DH + 1, :], in_=av_acc[h][DH : DH + 1, :]
                    )
                    ps_rc = ps_sim.tile([DH, NQ], F32, tag="ps", name=f"psrc{h}")
                    nc.tensor.matmul(
                        out=ps_rc[:],
                        lhsT=ones_t[DH : DH + 1, 0:DH],
                        rhs=rc_sb[DH : DH + 1, :],
                        start=True,
                        stop=True,
                    )
                    ot = p_ot.tile([DH, NQ], BF16, tag="ot", name=f"ot{h}")
                    nc.vector.tensor_tensor(
                        out=ot[:],
                        in0=av_acc[h][0:DH, :],
                        in1=ps_rc[:],
                        op=AX.mult,
                    )
                    ot_tiles.append(ot)

    # wout reuses the wp slot (projections no longer need the kv weights)
    wout = p_wp.tile([DH, N_HEAD, D_MODEL], BF16, tag="wp")
    nc.gpsimd.dma_start(out=wout[:], in_=wout_d[:])

    # --- phase D: output projection --------------------------------------
    for q2 in range(NQ // 128):
        fin = p_fin.tile([128, D_MODEL], F32, tag="fin", name=f"fin{q2}")
        for n2 in range(2):
            psf = ps_kv.tile([128, 384], F32, tag="ps", name=f"pf{q2}_{n2}")
            for h in range(N_HEAD):
                nc.tensor.matmul(
                    out=psf[:],
                    lhsT=ot_tiles[h][:, q2 * 128 : (q2 + 1) * 128],
                    rhs=wout[:, h, n2 * 384 : (n2 + 1) * 384],
                    start=(h == 0),
                    stop=(h == N_HEAD - 1),
                )
            nc.vector.tensor_tensor(
                out=fin[:, n2 * 384 : (n2 + 1) * 384],
                in0=psf[:],
                in1=rrep[:, n2 * 384 : (n2 + 1) * 384],
                op=AX.add,
            )
        nc.sync.dma_start(out=out_d[q2 * 128 : (q2 + 1) * 128, :], in_=fin[:])


def build_nc(reps=1):
    nc = bacc.Bacc(
        "TRN2", target_bir_lowering=False, debug=False, num_devices=N_CORES
    )
    x_d = nc.dram_tensor("x", [N_TOK, D_CTX], F32, kind="ExternalInput").ap()
    wp_d = nc.dram_tensor("wp", [D_CTX, 2 * INNER], BF16, kind="ExternalInput").ap()
    qt_d = nc.dram_tensor("qt", [INNER, NQ], BF16, kind="ExternalInput").ap()
    wout_d = nc.dram_tensor(
        "wout", [DH, N_HEAD, D_MODEL], BF16, kind="ExternalInput"
    ).ap()
    rrep_d = nc.dram_tensor("rrep", [128, D_MODEL], F32, kind="ExternalInput").ap()
    out_d = nc.dram_tensor("out", [NQ, D_MODEL], F32, kind="ExternalOutput").ap()
    from contextlib import ExitStack

    with tile.TileContext(nc) as tc:
        for rep in range(reps):
            with ExitStack() as ctx:
                emit_kernel(ctx, tc, out_d, x_d, wp_d, qt_d, wout_d, rrep_d, rep=rep)
    nc.compile()
    return nc


def host_prep(query, ln_q_w, ln_q_b, ln_k_w, ln_k_b, Wq, Wkv, Wout):
    """Batch-independent fp32 preprocessing. Returns per-core input dict
    (minus x)."""
    query = np.asarray(query, np.float32)
    mu = query.mean(-1, keepdims=True)
    var = ((query - mu) ** 2).mean(-1, keepdims=True)
    qn = (query - mu) / np.sqrt(var + EPS) * ln_q_w + ln_q_b
    qmat = (qn @ np.asarray(Wq, np.float32)) * (DH**-0.5)  # [NQ, INNER]
    qT = np.ascontiguousarray(qmat.T).astype(ml_dtypes.bfloat16)

    Wkv = np.asarray(Wkv, np.float32)
    Wp = (np.asarray(ln_k_w, np.float32)[:, None] * Wkv).astype(ml_dtypes.bfloat16)
    c = np.asarray(ln_k_b, np.float32) @ Wkv  # [2*INNER]
    c_v = c[INNER:]
    Wout = np.asarray(Wout, np.float32)
    r = c_v @ Wout  # [D_MODEL]
    rrep = np.ascontiguousarray(np.broadcast_to(r, (128, D_MODEL))).astype(np.float32)
    wout_arr = np.ascontiguousarray(
        Wout.reshape(N_HEAD, DH, D_MODEL).transpose(1, 0, 2)
    ).astype(ml_dtypes.bfloat16)
    return {"wp": Wp, "qt": qT, "wout": wout_arr, "rrep": rrep}


_NC_CACHE = {}


def get_nc():
    if "nc" not in _NC_CACHE:
        _NC_CACHE["nc"] = build_nc()
    return _NC_CACHE["nc"]


def kernel(x, query, ln_q_w, ln_q_b, ln_k_w, ln_k_b, Wq, Wkv, Wout):
    x = np.asarray(x, np.float32)
    shared = host_prep(query, ln_q_w, ln_q_b, ln_k_w, ln_k_b, Wq, Wkv, Wout)
    in_maps = [
        {"x": np.ascontiguousarray(x[b]), **shared} for b in range(B)
    ]
    nc = get_nc()
    res = run_bass_kernel_spmd(nc, in_maps, list(range(N_CORES)))
    return np.stack([res.results[b]["out"] for b in range(B)], axis=0)


# revision 44
# speedup vs baseline: 1.0336x; 1.0036x over previous
"""AttentionalPooler Trainium2 kernel.

Full inputs -> full output; batch (8) is data-parallel across the 8
NeuronCores. Per core: LayerNorm(x_b), kv = LN(x_b) @ Wkv, 12-head
cross-attention from 256 pre-computed queries, output projection.

Host-side preprocessing (exact fp32 algebra, batch-independent):
  - q path (LN(query) @ Wq * dh^-0.5, transposed) is computed on host.
  - ln_k_w is folded into the kv weights (Wp = diag(ln_k_w) @ Wkv).
  - ln_k_b folds into c = ln_k_b @ Wkv. The k-part of c shifts every
    logit of a (head, query) row by the same constant, which softmax
    cancels exactly, so it is dropped. The v-part adds c_v to every
    attention output row (attention weights sum to 1), so it commutes
    past Wout: the kernel adds r = c_v @ Wout to the final output.

Device schedule (single pass over token chunks; small leading chunks
shrink the pipeline-fill bubble and dummy matmuls keep the PE HAM
clock-gate warm through it):
  - Per chunk: cast-load x tiles to bf16, LayerNorm on DVE, bounce the
    normalized tiles through DRAM, one large xbar DMA-transpose per
    d-tile into xnT, then: kT projection matmuls (e-major) -> sim
    matmuls for all 6 head pairs (simT[tok, query], K=64 row-pair
    packed) -> exp on ACT -> V projection matmuls -> attn@v chunk
    matmuls accumulated into per-head SBUF accumulators. Emitting sim
    before v/attn@v lets the ACT exp hide under PE work.
  - Softmax denominators come from a ones-column appended to v; max-
    subtraction is skipped (logits provably small for LN'd inputs).
  - Tail: per-head normalize (reciprocal partition-broadcast via a K=1
    ones-matmul) and the output projection (+ the c_v@Wout constant).
"""

import sys

sys.path.insert(0, "/opt/trn_rl_repo")

import numpy as np
import ml_dtypes

import concourse.bass as bass
import concourse.mybir as mybir
import concourse.tile as tile
from concourse import bacc
from concourse.bass_utils import run_bass_kernel_spmd

F32 = mybir.dt.float32
BF16 = mybir.dt.bfloat16
AX = mybir.AluOpType

B = 8
N_TOK = 4096
D_CTX = 1024
D_MODEL = 768
N_HEAD = 12
DH = 64
NQ = 256
INNER = 768
EPS = 1e-5
N_CORES = 8

TOK_TILES = N_TOK // 128  # 32
D_TILES = D_CTX // 128  # 8
E_TILES = INNER // 128  # 6


def emit_kernel(ctx, tc, out_d, x_d, wp_d, qt_d, wout_d, rrep_d, rep=0):
    nc = tc.nc
    xn_dram = nc.dram_tensor(f"xn_scratch{rep}", [N_TOK, D_CTX], BF16).ap()

    p_wp = ctx.enter_context(tc.tile_pool(name="wp", bufs=1))
    p_qt = ctx.enter_context(tc.tile_pool(name="qt", bufs=1))
    p_r = ctx.enter_context(tc.tile_pool(name="rr", bufs=1))
    p_x = ctx.enter_context(tc.tile_pool(name="x", bufs=3))
    p_xn = ctx.enter_context(tc.tile_pool(name="xn", bufs=2))
    p_big = ctx.enter_context(tc.tile_pool(name="big", bufs=2))
    p_attn = ctx.enter_context(tc.tile_pool(name="attn", bufs=3))
    p_kt = ctx.enter_context(tc.tile_pool(name="kt", bufs=E_TILES))
    p_v = ctx.enter_context(tc.tile_pool(name="v", bufs=TOK_TILES))
    p_acc = ctx.enter_context(tc.tile_pool(name="acc", bufs=N_HEAD))
    p_stat = ctx.enter_context(tc.tile_pool(name="stat", bufs=4))
    p_ot = ctx.enter_context(tc.tile_pool(name="ot", bufs=N_HEAD))
    p_fin = ctx.enter_context(tc.tile_pool(name="fin", bufs=1))
    p_rc = ctx.enter_context(tc.tile_pool(name="rc", bufs=2))
    ps_kv = ctx.enter_context(tc.tile_pool(name="pskv", bufs=2, space="PSUM"))
    ps_sim = ctx.enter_context(tc.tile_pool(name="pssim", bufs=2, space="PSUM"))
    ps_av = ctx.enter_context(tc.tile_pool(name="psav", bufs=2, space="PSUM"))

    # --- LN(x) preprocessing for one x-tile ------------------------------
    def prep_tile(i):
        xt = p_x.tile([128, D_CTX], BF16, tag="x", name=f"x{i}")
        nc.sync.dma_start(out=xt[:], in_=x_d[i * 128 : (i + 1) * 128, :])

        st = p_stat.tile([128, 2, 6], F32, tag="st", name=f"st{i}")
        nc.vector.bn_stats(out=st[:, 0, :], in_=xt[:, 0:512])
        nc.vector.bn_stats(out=st[:, 1, :], in_=xt[:, 512:1024])
        mv = p_stat.tile([128, 2], F32, tag="mv", name=f"mv{i}")
        nc.vector.bn_aggr(out=mv[:], in_=st[:])
        rstd = p_stat.tile([128, 1], F32, tag="rstd", name=f"rstd{i}")
        nc.scalar.activation(
            out=rstd[:],
            in_=mv[:, 1:2],
            func=mybir.ActivationFunctionType.Sqrt,
            bias=eps_t[:],
            scale=1.0,
        )
        nc.vector.reciprocal(out=rstd[:], in_=rstd[:])
        negmr = p_stat.tile([128, 1], F32, tag="negmr", name=f"negmr{i}")
        nc.vector.scalar_tensor_tensor(
            out=negmr[:],
            in0=mv[:, 0:1],
            scalar=-1.0,
            in1=rstd[:],
            op0=AX.mult,
            op1=AX.mult,
        )
        xn = p_xn.tile([128, D_CTX], BF16, tag="xn", name=f"xn{i}")
        nc.vector.tensor_scalar(
            out=xn[:],
            in0=xt[:],
            scalar1=rstd[:, 0:1],
            scalar2=negmr[:, 0:1],
            op0=AX.mult,
            op1=AX.add,
        )
        nc.sync.dma_start(out=xn_dram[i * 128 : (i + 1) * 128, :], in_=xn[:])

    # --- get the x pipeline going before the big weight loads ------------
    p_eps = ctx.enter_context(tc.tile_pool(name="eps", bufs=1))
    eps_t = p_eps.tile([128, 1], F32, tag="eps")
    nc.vector.memset(eps_t[:], EPS)
    for i in range(6):
        prep_tile(i)

    # --- constant loads (after the quarter-0 x pipeline is in flight).
    # Split the kv weights into k/v halves so the first kT matmuls only
    # wait on the k half.
    wp = p_wp.tile([128, D_TILES, 2 * INNER], BF16, tag="wp")
    wp_r = wp_d.rearrange("(t p) n -> p t n", p=128)
    nc.gpsimd.dma_start(out=wp[:, :, 0:INNER], in_=wp_r[:, :, 0:INNER])
    qt = p_qt.tile([128, E_TILES, NQ], BF16)
    nc.gpsimd.dma_start(out=qt[:], in_=qt_d.rearrange("(t p) n -> p t n", p=128))
    nc.gpsimd.dma_start(
        out=wp[:, :, INNER : 2 * INNER], in_=wp_r[:, :, INNER : 2 * INNER]
    )

    kt_tiles = []
    for e in range(E_TILES):
        kt_tiles.append(p_kt.tile([128, N_TOK], BF16, tag="kt", name=f"kt{e}"))
    v_tiles = []
    for j in range(TOK_TILES):
        v_tiles.append(p_v.tile([128, N_HEAD, DH + 1], BF16, tag="v", name=f"v{j}"))

    av_acc = []
    for h in range(N_HEAD):
        av_acc.append(p_acc.tile([DH + 1, NQ], F32, tag="acc", name=f"acc{h}"))
    ot_tiles = []
    rrep = p_r.tile([128, D_MODEL], F32)
    nc.gpsimd.dma_start(out=rrep[:], in_=rrep_d[:])
    ones_t = p_r.tile([128, DH], F32, tag="ones")
    nc.vector.memset(ones_t[:], 1.0)

    # PE warm-up: ~3.5us of dummy matmuls during the pipeline-fill window
    # so the HAM clock gate is at full rate when the real work arrives.
    warm = p_r.tile([128, 256], BF16, tag="warm")
    nc.vector.memset(warm[:], 1.0)
    wps = ps_sim.tile([128, 4, NQ], F32, tag="ps", name="warmps")
    for _ in range(36):
        nc.tensor.matmul(
            out=wps.rearrange("p a b -> p (a b)")[:, 0:256],
            lhsT=warm[:, 0:128],
            rhs=warm[:],
            start=True,
            stop=True,
        )

    # --- main pass: per quarter: LN -> xnT -> kT -> sim -> v -> attn@v ---
    # small leading quarters cut the pipeline-fill bubble before the
    # first matmul
    qsizes = [2, 2, 4, 8, 8, 8]
    qstarts = [sum(qsizes[:g]) for g in range(len(qsizes))]
    prepped = 6
    for q, (j0, nj) in enumerate(zip(qstarts, qsizes)):
        last_q = q == len(qsizes) - 1
        for _ in range(nj):
            if prepped < TOK_TILES:
                prep_tile(prepped)
                prepped += 1
        xnt = p_big.tile([128, D_TILES, nj * 128], BF16, tag="big",
                         name=f"xnt{q}")
        for d in range(D_TILES):
            nc.sync.dma_start(
                out=xnt[:, d, :],
                in_=xn_dram[j0 * 128 : (j0 + nj) * 128, d * 128 : (d + 1) * 128],
                transpose=True,
            )
        for e in range(E_TILES):
            for n2 in range(max(1, nj * 128 // 512)):
                nw = min(512, nj * 128)
                ps = ps_kv.tile([128, 512], F32, tag="ps", name=f"pkt{q}_{e}_{n2}")
                for d in range(D_TILES):
                    nc.tensor.matmul(
                        out=ps[:, 0:nw],
                        lhsT=wp[:, d, e * 128 : (e + 1) * 128],
                        rhs=xnt[:, d, n2 * 512 : n2 * 512 + nw],
                        start=(d == 0),
                        stop=(d == D_TILES - 1),
                    )
                c0 = j0 * 128 + n2 * 512
                nc.vector.tensor_copy(
                    out=kt_tiles[e][:, c0 : c0 + nw], in_=ps[:, 0:nw]
                )

        # sim + exp for all pairs over this quarter's token tiles
        attn_tiles = {}
        for p in range(E_TILES):
            for hh, base in ((0, 0), (1, 64)):
                attn = p_attn.tile([128, nj, NQ], BF16, tag="attn",
                                   name=f"at{q}_{p}_{hh}")
                attn_tiles[(p, hh)] = attn
                for g0 in range(0, nj, 4):
                    ng = min(4, nj - g0)
                    ps = ps_sim.tile([128, 4, NQ], F32, tag="ps",
                                     name=f"psim{q}_{p}_{hh}_{g0}")
                    for jj in range(ng):
                        j = j0 + g0 + jj
                        nc.tensor.matmul(
                            out=ps[:, jj, :],
                            lhsT=kt_tiles[p][base : base + 64,
                                             j * 128 : (j + 1) * 128],
                            rhs=qt[base : base + 64, p, :],
                            start=True,
                            stop=True,
                        )
                    nc.scalar.activation(
                        out=attn[:, g0 : g0 + ng, :],
                        in_=ps[:, 0:ng, :],
                        func=mybir.ActivationFunctionType.Exp,
                    )

        # v projection for this quarter
        for jj in range(nj):
            j = j0 + jj
            vt = v_tiles[j]
            nc.vector.memset(vt[:, :, DH : DH + 1], 1.0)
            for h6 in range(2):
                ps = ps_kv.tile([128, 384], F32, tag="ps", name=f"pv{j}_{h6}")
                for d in range(D_TILES):
                    nc.tensor.matmul(
                        out=ps[:],
                        lhsT=xnt[:, d, jj * 128 : (jj + 1) * 128],
                        rhs=wp[:, d, INNER + h6 * 384 : INNER + (h6 + 1) * 384],
                        start=(d == 0),
                        stop=(d == D_TILES - 1),
                    )
                nc.vector.tensor_copy(
                    out=vt[:, h6 * 6 : (h6 + 1) * 6, 0:DH],
                    in_=ps.rearrange("p (h dh) -> p h dh", dh=DH),
                )

        # attn@v chunks for all pairs
        for p in range(E_TILES):
            for hh in range(2):
                h = 2 * p + hh
                attn = attn_tiles[(p, hh)]
                psa = ps_av.tile([DH + 1, NQ], F32, tag="ps", name=f"pav{q}_{h}")
                for jj in range(nj):
                    nc.tensor.matmul(
                        out=psa[:],
                        lhsT=v_tiles[j0 + jj][:, h, :],
                        rhs=attn[:, jj, :],
                        start=(jj == 0),
                        stop=(jj == nj - 1),
                    )
                if q == 0:
                    nc.vector.tensor_copy(out=av_acc[h][:], in_=psa[:])
                else:
                    nc.vector.tensor_tensor(
                        out=av_acc[h][:], in0=av_acc[h][:], in1=psa[:], op=AX.add
                    )
                if last_q:
                    # phase C inline: per-head softmax normalize as soon as
                    # the last chunk lands. The reciprocal is partition-
                    # broadcast with a K=1 ones-matmul into PSUM.
                    rc_sb = p_rc.tile([128, NQ], F32, tag="rcsb", name=f"rc{h}")
                    nc.vector.reciprocal(
                        out=rc_sb[DH : DH + 1, :], in_=av_acc[h][DH : DH + 1, :]
                    )
                    ps_rc = ps_sim.tile([DH, NQ], F32, tag="ps", name=f"psrc{h}")
                    nc.tensor.matmul(
                        out=ps_rc[:],
                        lhsT=ones_t[DH : DH + 1, 0:DH],
                        rhs=rc_sb[DH : DH + 1, :],
                        start=True,
                        stop=True,
                    )
                    ot = p_ot.tile([DH, NQ], BF16, tag="ot", name=f"ot{h}")
                    nc.vector.tensor_tensor(
                        out=ot[:],
                        in0=av_acc[h][0:DH, :],
                        in1=ps_rc[:],
                        op=AX.mult,
                    )
                    ot_tiles.append(ot)

    # wout reuses the wp slot (projections no longer need the kv weights)
    wout = p_wp.tile([DH, N_HEAD, D_MODEL], BF16, tag="wp")
    nc.gpsimd.dma_start(out=wout[:], in_=wout_d[:])

    # --- phase D: output projection --------------------------------------
    for q2 in range(NQ // 128):
        fin = p_fin.tile([128, D_MODEL], F32, tag="fin", name=f"fin{q2}")
        for n2 in range(2):
            psf = ps_kv.tile([128, 384], F32, tag="ps", name=f"pf{q2}_{n2}")
            for h in range(N_HEAD):
                nc.tensor.matmul(
                    out=psf[:],
                    lhsT=ot_tiles[h][:, q2 * 128 : (q2 + 1) * 128],
                    rhs=wout[:, h, n2 * 384 : (n2 + 1) * 384],
                    start=(h == 0),
                    stop=(h == N_HEAD - 1),
                )
            nc.vector.tensor_tensor(
                out=fin[:, n2 * 384 : (n2 + 1) * 384],
                in0=psf[:],
                in1=rrep[:, n2 * 384 : (n2 + 1) * 384],
                op=AX.add,
            )
        nc.sync.dma_start(out=out_d[q2 * 128 : (q2 + 1) * 128, :], in_=fin[:])


def build_nc(reps=1):
    nc = bacc.Bacc(
        "TRN2", target_bir_lowering=False, debug=False, num_devices=N_CORES
    )
    x_d = nc.dram_tensor("x", [N_TOK, D_CTX], BF16, kind="ExternalInput").ap()
    wp_d = nc.dram_tensor("wp", [D_CTX, 2 * INNER], BF16, kind="ExternalInput").ap()
    qt_d = nc.dram_tensor("qt", [INNER, NQ], BF16, kind="ExternalInput").ap()
    wout_d = nc.dram_tensor(
        "wout", [DH, N_HEAD, D_MODEL], BF16, kind="ExternalInput"
    ).ap()
    rrep_d = nc.dram_tensor("rrep", [128, D_MODEL], F32, kind="ExternalInput").ap()
    out_d = nc.dram_tensor("out", [NQ, D_MODEL], F32, kind="ExternalOutput").ap()
    from contextlib import ExitStack

    with tile.TileContext(nc) as tc:
        for rep in range(reps):
            with ExitStack() as ctx:
                emit_kernel(ctx, tc, out_d, x_d, wp_d, qt_d, wout_d, rrep_d, rep=rep)
    nc.compile()
    return nc


def host_prep(query, ln_q_w, ln_q_b, ln_k_w, ln_k_b, Wq, Wkv, Wout):
    """Batch-independent fp32 preprocessing. Returns per-core input dict
    (minus x)."""
    query = np.asarray(query, np.float32)
    mu = query.mean(-1, keepdims=True)
    var = ((query - mu) ** 2).mean(-1, keepdims=True)
    qn = (query - mu) / np.sqrt(var + EPS) * ln_q_w + ln_q_b
    qmat = (qn @ np.asarray(Wq, np.float32)) * (DH**-0.5)  # [NQ, INNER]
    qT = np.ascontiguousarray(qmat.T).astype(ml_dtypes.bfloat16)

    Wkv = np.asarray(Wkv, np.float32)
    Wp = (np.asarray(ln_k_w, np.float32)[:, None] * Wkv).astype(ml_dtypes.bfloat16)
    c = np.asarray(ln_k_b, np.float32) @ Wkv  # [2*INNER]
    c_v = c[INNER:]
    Wout = np.asarray(Wout, np.float32)
    r = c_v @ Wout  # [D_MODEL]
    rrep = np.ascontiguousarray(np.broadcast_to(r, (128, D_MODEL))).astype(np.float32)
    wout_arr = np.ascontiguousarray(
        Wout.reshape(N_HEAD, DH, D_MODEL).transpose(1, 0, 2)
    ).astype(ml_dtypes.bfloat16)
    return {"wp": Wp, "qt": qT, "wout": wout_arr, "rrep": rrep}


_NC_CACHE = {}


def get_nc():
    if "nc" not in _NC_CACHE:
        _NC_CACHE["nc"] = build_nc()
    return _NC_CACHE["nc"]


def kernel(x, query, ln_q_w, ln_q_b, ln_k_w, ln_k_b, Wq, Wkv, Wout):
    x = np.asarray(x, np.float32)
    shared = host_prep(query, ln_q_w, ln_q_b, ln_k_w, ln_k_b, Wq, Wkv, Wout)
    in_maps = [
        {"x": np.ascontiguousarray(x[b]).astype(ml_dtypes.bfloat16), **shared}
        for b in range(B)
    ]
    nc = get_nc()
    res = run_bass_kernel_spmd(nc, in_maps, list(range(N_CORES)))
    return np.stack([res.results[b]["out"] for b in range(B)], axis=0)


# revision 45
# speedup vs baseline: 1.0644x; 1.0298x over previous
"""AttentionalPooler Trainium2 kernel.

Full inputs -> full output; batch (8) is data-parallel across the 8
NeuronCores. Per core: LayerNorm(x_b), kv = LN(x_b) @ Wkv, 12-head
cross-attention from 256 pre-computed queries, output projection.

Host-side preprocessing (exact fp32 algebra, batch-independent):
  - q path (LN(query) @ Wq * dh^-0.5, transposed) is computed on host.
  - ln_k_w is folded into the kv weights (Wp = diag(ln_k_w) @ Wkv).
  - ln_k_b folds into c = ln_k_b @ Wkv. The k-part of c shifts every
    logit of a (head, query) row by the same constant, which softmax
    cancels exactly, so it is dropped. The v-part adds c_v to every
    attention output row (attention weights sum to 1), so it commutes
    past Wout: the kernel adds r = c_v @ Wout to the final output.

Device schedule (single pass over token chunks; small leading chunks
shrink the pipeline-fill bubble and dummy matmuls keep the PE HAM
clock-gate warm through it):
  - Per chunk: cast-load x tiles to bf16, LayerNorm on DVE, bounce the
    normalized tiles through DRAM, one large xbar DMA-transpose per
    d-tile into xnT, then: kT projection matmuls (e-major) -> sim
    matmuls for all 6 head pairs (simT[tok, query], K=64 row-pair
    packed) -> exp on ACT -> V projection matmuls -> attn@v chunk
    matmuls accumulated into per-head SBUF accumulators. Emitting sim
    before v/attn@v lets the ACT exp hide under PE work.
  - Softmax denominators come from a ones-column appended to v; max-
    subtraction is skipped (logits provably small for LN'd inputs).
  - Tail: per-head normalize (reciprocal partition-broadcast via a K=1
    ones-matmul) and the output projection (+ the c_v@Wout constant).
"""

import sys

sys.path.insert(0, "/opt/trn_rl_repo")

import numpy as np
import ml_dtypes

import concourse.bass as bass
import concourse.mybir as mybir
import concourse.tile as tile
from concourse import bacc
from concourse.bass_utils import run_bass_kernel_spmd

F32 = mybir.dt.float32
BF16 = mybir.dt.bfloat16
AX = mybir.AluOpType

B = 8
N_TOK = 4096
D_CTX = 1024
D_MODEL = 768
N_HEAD = 12
DH = 64
NQ = 256
INNER = 768
EPS = 1e-5
N_CORES = 8

TOK_TILES = N_TOK // 128  # 32
D_TILES = D_CTX // 128  # 8
E_TILES = INNER // 128  # 6


def emit_kernel(ctx, tc, out_d, x_d, wp_d, qt_d, wout_d, rrep_d, rep=0):
    nc = tc.nc
    xn_dram = nc.dram_tensor(f"xn_scratch{rep}", [N_TOK, D_CTX], BF16).ap()

    p_wp = ctx.enter_context(tc.tile_pool(name="wp", bufs=1))
    p_qt = ctx.enter_context(tc.tile_pool(name="qt", bufs=1))
    p_r = ctx.enter_context(tc.tile_pool(name="rr", bufs=1))
    p_x = ctx.enter_context(tc.tile_pool(name="x", bufs=3))
    p_xn = ctx.enter_context(tc.tile_pool(name="xn", bufs=2))
    p_big = ctx.enter_context(tc.tile_pool(name="big", bufs=2))
    p_attn = ctx.enter_context(tc.tile_pool(name="attn", bufs=3))
    p_kt = ctx.enter_context(tc.tile_pool(name="kt", bufs=E_TILES))
    p_v = ctx.enter_context(tc.tile_pool(name="v", bufs=TOK_TILES))
    p_acc = ctx.enter_context(tc.tile_pool(name="acc", bufs=N_HEAD))
    p_stat = ctx.enter_context(tc.tile_pool(name="stat", bufs=4))
    p_ot = ctx.enter_context(tc.tile_pool(name="ot", bufs=N_HEAD))
    p_fin = ctx.enter_context(tc.tile_pool(name="fin", bufs=1))
    p_rc = ctx.enter_context(tc.tile_pool(name="rc", bufs=2))
    ps_kv = ctx.enter_context(tc.tile_pool(name="pskv", bufs=2, space="PSUM"))
    ps_sim = ctx.enter_context(tc.tile_pool(name="pssim", bufs=4, space="PSUM"))
    ps_av = ctx.enter_context(tc.tile_pool(name="psav", bufs=2, space="PSUM"))

    # --- LN(x) preprocessing for one x-tile ------------------------------
    def prep_tile(i):
        xt = p_x.tile([128, D_CTX], BF16, tag="x", name=f"x{i}")
        nc.sync.dma_start(out=xt[:], in_=x_d[i * 128 : (i + 1) * 128, :])

        st = p_stat.tile([128, 2, 6], F32, tag="st", name=f"st{i}")
        nc.vector.bn_stats(out=st[:, 0, :], in_=xt[:, 0:512])
        nc.vector.bn_stats(out=st[:, 1, :], in_=xt[:, 512:1024])
        mv = p_stat.tile([128, 2], F32, tag="mv", name=f"mv{i}")
        nc.vector.bn_aggr(out=mv[:], in_=st[:])
        rstd = p_stat.tile([128, 1], F32, tag="rstd", name=f"rstd{i}")
        nc.scalar.activation(
            out=rstd[:],
            in_=mv[:, 1:2],
            func=mybir.ActivationFunctionType.Sqrt,
            bias=eps_t[:],
            scale=1.0,
        )
        nc.vector.reciprocal(out=rstd[:], in_=rstd[:])
        negmr = p_stat.tile([128, 1], F32, tag="negmr", name=f"negmr{i}")
        nc.vector.scalar_tensor_tensor(
            out=negmr[:],
            in0=mv[:, 0:1],
            scalar=-1.0,
            in1=rstd[:],
            op0=AX.mult,
            op1=AX.mult,
        )
        xn = p_xn.tile([128, D_CTX], BF16, tag="xn", name=f"xn{i}")
        nc.vector.tensor_scalar(
            out=xn[:],
            in0=xt[:],
            scalar1=rstd[:, 0:1],
            scalar2=negmr[:, 0:1],
            op0=AX.mult,
            op1=AX.add,
        )
        nc.sync.dma_start(out=xn_dram[i * 128 : (i + 1) * 128, :], in_=xn[:])

    # --- get the x pipeline going before the big weight loads ------------
    p_eps = ctx.enter_context(tc.tile_pool(name="eps", bufs=1))
    eps_t = p_eps.tile([128, 1], F32, tag="eps")
    nc.vector.memset(eps_t[:], EPS)
    for i in range(6):
        prep_tile(i)

    # --- constant loads (after the quarter-0 x pipeline is in flight).
    # Split the kv weights into k/v halves so the first kT matmuls only
    # wait on the k half.
    wp = p_wp.tile([128, D_TILES, 2 * INNER], BF16, tag="wp")
    wp_r = wp_d.rearrange("(t p) n -> p t n", p=128)
    nc.gpsimd.dma_start(out=wp[:, :, 0:INNER], in_=wp_r[:, :, 0:INNER])
    qt = p_qt.tile([128, E_TILES, NQ], BF16)
    nc.gpsimd.dma_start(out=qt[:], in_=qt_d.rearrange("(t p) n -> p t n", p=128))
    nc.gpsimd.dma_start(
        out=wp[:, :, INNER : 2 * INNER], in_=wp_r[:, :, INNER : 2 * INNER]
    )

    kt_tiles = []
    for e in range(E_TILES):
        kt_tiles.append(p_kt.tile([128, N_TOK], BF16, tag="kt", name=f"kt{e}"))
    v_tiles = []
    for j in range(TOK_TILES):
        v_tiles.append(p_v.tile([128, N_HEAD, DH + 1], BF16, tag="v", name=f"v{j}"))

    av_acc = []
    for h in range(N_HEAD):
        av_acc.append(p_acc.tile([DH + 1, NQ], F32, tag="acc", name=f"acc{h}"))
    ot_tiles = []
    rrep = p_r.tile([128, D_MODEL], F32)
    nc.gpsimd.dma_start(out=rrep[:], in_=rrep_d[:])
    ones_t = p_r.tile([128, DH], F32, tag="ones")
    nc.vector.memset(ones_t[:], 1.0)

    # PE warm-up: ~3.5us of dummy matmuls during the pipeline-fill window
    # so the HAM clock gate is at full rate when the real work arrives.
    warm = p_r.tile([128, 256], BF16, tag="warm")
    nc.vector.memset(warm[:], 1.0)
    wps = ps_sim.tile([128, 2, NQ], F32, tag="ps", name="warmps")
    for _ in range(36):
        nc.tensor.matmul(
            out=wps.rearrange("p a b -> p (a b)")[:, 0:256],
            lhsT=warm[:, 0:128],
            rhs=warm[:],
            start=True,
            stop=True,
        )

    # --- main pass: per quarter: LN -> xnT -> kT -> sim -> v -> attn@v ---
    # small leading quarters cut the pipeline-fill bubble before the
    # first matmul
    qsizes = [2, 2, 4, 8, 8, 8]
    qstarts = [sum(qsizes[:g]) for g in range(len(qsizes))]
    prepped = 6
    for q, (j0, nj) in enumerate(zip(qstarts, qsizes)):
        last_q = q == len(qsizes) - 1
        for _ in range(nj):
            if prepped < TOK_TILES:
                prep_tile(prepped)
                prepped += 1
        xnt = p_big.tile([128, D_TILES, nj * 128], BF16, tag="big",
                         name=f"xnt{q}")
        for d in range(D_TILES):
            nc.sync.dma_start(
                out=xnt[:, d, :],
                in_=xn_dram[j0 * 128 : (j0 + nj) * 128, d * 128 : (d + 1) * 128],
                transpose=True,
            )
        for e in range(E_TILES):
            for n2 in range(max(1, nj * 128 // 512)):
                nw = min(512, nj * 128)
                ps = ps_kv.tile([128, 512], F32, tag="ps", name=f"pkt{q}_{e}_{n2}")
                for d in range(D_TILES):
                    nc.tensor.matmul(
                        out=ps[:, 0:nw],
                        lhsT=wp[:, d, e * 128 : (e + 1) * 128],
                        rhs=xnt[:, d, n2 * 512 : n2 * 512 + nw],
                        start=(d == 0),
                        stop=(d == D_TILES - 1),
                    )
                c0 = j0 * 128 + n2 * 512
                nc.vector.tensor_copy(
                    out=kt_tiles[e][:, c0 : c0 + nw], in_=ps[:, 0:nw]
                )

        # sim + exp for all pairs over this quarter's token tiles.
        # Groups of 2 token tiles; the head-A (rows 0:64) and head-B
        # (rows 64:128) matmul batches of a group are adjacent so the PE
        # runs the disjoint row-groups concurrently, and 4 single-bank
        # sim psum slots keep the pipeline from stalling on ACT.
        attn_tiles = {}
        for p in range(E_TILES):
            for hh in range(2):
                attn_tiles[(p, hh)] = p_attn.tile(
                    [128, nj, NQ], BF16, tag="attn", name=f"at{q}_{p}_{hh}"
                )
            for g0 in range(0, nj, 2):
                ng = min(2, nj - g0)
                for hh, base in ((0, 0), (1, 64)):
                    ps = ps_sim.tile([128, 2, NQ], F32, tag="ps",
                                     name=f"psim{q}_{p}_{hh}_{g0}")
                    for jj in range(ng):
                        j = j0 + g0 + jj
                        nc.tensor.matmul(
                            out=ps[:, jj, :],
                            lhsT=kt_tiles[p][base : base + 64,
                                             j * 128 : (j + 1) * 128],
                            rhs=qt[base : base + 64, p, :],
                            start=True,
                            stop=True,
                        )
                    nc.scalar.activation(
                        out=attn_tiles[(p, hh)][:, g0 : g0 + ng, :],
                        in_=ps[:, 0:ng, :],
                        func=mybir.ActivationFunctionType.Exp,
                    )

        # v projection for this quarter
        for jj in range(nj):
            j = j0 + jj
            vt = v_tiles[j]
            nc.vector.memset(vt[:, :, DH : DH + 1], 1.0)
            for h6 in range(2):
                ps = ps_kv.tile([128, 384], F32, tag="ps", name=f"pv{j}_{h6}")
                for d in range(D_TILES):
                    nc.tensor.matmul(
                        out=ps[:],
                        lhsT=xnt[:, d, jj * 128 : (jj + 1) * 128],
                        rhs=wp[:, d, INNER + h6 * 384 : INNER + (h6 + 1) * 384],
                        start=(d == 0),
                        stop=(d == D_TILES - 1),
                    )
                nc.vector.tensor_copy(
                    out=vt[:, h6 * 6 : (h6 + 1) * 6, 0:DH],
                    in_=ps.rearrange("p (h dh) -> p h dh", dh=DH),
                )

        # attn@v chunks for all pairs
        for p in range(E_TILES):
            for hh in range(2):
                h = 2 * p + hh
                attn = attn_tiles[(p, hh)]
                psa = ps_av.tile([DH + 1, NQ], F32, tag="ps", name=f"pav{q}_{h}")
                for jj in range(nj):
                    nc.tensor.matmul(
                        out=psa[:],
                        lhsT=v_tiles[j0 + jj][:, h, :],
                        rhs=attn[:, jj, :],
                        start=(jj == 0),
                        stop=(jj == nj - 1),
                    )
                if q == 0:
                    nc.vector.tensor_copy(out=av_acc[h][:], in_=psa[:])
                else:
                    nc.vector.tensor_tensor(
                        out=av_acc[h][:], in0=av_acc[h][:], in1=psa[:], op=AX.add
                    )
                if last_q:
                    # phase C inline: per-head softmax normalize as soon as
                    # the last chunk lands. The reciprocal is partition-
                    # broadcast with a K=1 ones-matmul into PSUM.
                    rc_sb = p_rc.tile([128, NQ], F32, tag="rcsb", name=f"rc{h}")
                    nc.vector.reciprocal(
                        out=rc_sb[DH : DH + 1, :], in_=av_acc[h][DH : DH + 1, :]
                    )
                    ps_rc = ps_sim.tile([DH, NQ], F32, tag="ps", name=f"psrc{h}")
                    nc.tensor.matmul(
                        out=ps_rc[:],
                        lhsT=ones_t[DH : DH + 1, 0:DH],
                        rhs=rc_sb[DH : DH + 1, :],
                        start=True,
                        stop=True,
                    )
                    ot = p_ot.tile([DH, NQ], BF16, tag="ot", name=f"ot{h}")
                    nc.vector.tensor_tensor(
                        out=ot[:],
                        in0=av_acc[h][0:DH, :],
                        in1=ps_rc[:],
                        op=AX.mult,
                    )
                    ot_tiles.append(ot)

    # wout reuses the wp slot (projections no longer need the kv weights)
    wout = p_wp.tile([DH, N_HEAD, D_MODEL], BF16, tag="wp")
    nc.gpsimd.dma_start(out=wout[:], in_=wout_d[:])

    # --- phase D: output projection --------------------------------------
    for q2 in range(NQ // 128):
        fin = p_fin.tile([128, D_MODEL], F32, tag="fin", name=f"fin{q2}")
        for n2 in range(2):
            psf = ps_kv.tile([128, 384], F32, tag="ps", name=f"pf{q2}_{n2}")
            for h in range(N_HEAD):
                nc.tensor.matmul(
                    out=psf[:],
                    lhsT=ot_tiles[h][:, q2 * 128 : (q2 + 1) * 128],
                    rhs=wout[:, h, n2 * 384 : (n2 + 1) * 384],
                    start=(h == 0),
                    stop=(h == N_HEAD - 1),
                )
            nc.vector.tensor_tensor(
                out=fin[:, n2 * 384 : (n2 + 1) * 384],
                in0=psf[:],
                in1=rrep[:, n2 * 384 : (n2 + 1) * 384],
                op=AX.add,
            )
        nc.sync.dma_start(out=out_d[q2 * 128 : (q2 + 1) * 128, :], in_=fin[:])


def build_nc(reps=1):
    nc = bacc.Bacc(
        "TRN2", target_bir_lowering=False, debug=False, num_devices=N_CORES
    )
    x_d = nc.dram_tensor("x", [N_TOK, D_CTX], BF16, kind="ExternalInput").ap()
    wp_d = nc.dram_tensor("wp", [D_CTX, 2 * INNER], BF16, kind="ExternalInput").ap()
    qt_d = nc.dram_tensor("qt", [INNER, NQ], BF16, kind="ExternalInput").ap()
    wout_d = nc.dram_tensor(
        "wout", [DH, N_HEAD, D_MODEL], BF16, kind="ExternalInput"
    ).ap()
    rrep_d = nc.dram_tensor("rrep", [128, D_MODEL], F32, kind="ExternalInput").ap()
    out_d = nc.dram_tensor("out", [NQ, D_MODEL], F32, kind="ExternalOutput").ap()
    from contextlib import ExitStack

    with tile.TileContext(nc) as tc:
        for rep in range(reps):
            with ExitStack() as ctx:
                emit_kernel(ctx, tc, out_d, x_d, wp_d, qt_d, wout_d, rrep_d, rep=rep)
    nc.compile()
    return nc


def host_prep(query, ln_q_w, ln_q_b, ln_k_w, ln_k_b, Wq, Wkv, Wout):
    """Batch-independent fp32 preprocessing. Returns per-core input dict
    (minus x)."""
    query = np.asarray(query, np.float32)
    mu = query.mean(-1, keepdims=True)
    var = ((query - mu) ** 2).mean(-1, keepdims=True)
    qn = (query - mu) / np.sqrt(var + EPS) * ln_q_w + ln_q_b
    qmat = (qn @ np.asarray(Wq, np.float32)) * (DH**-0.5)  # [NQ, INNER]
    qT = np.ascontiguousarray(qmat.T).astype(ml_dtypes.bfloat16)

    Wkv = np.asarray(Wkv, np.float32)
    Wp = (np.asarray(ln_k_w, np.float32)[:, None] * Wkv).astype(ml_dtypes.bfloat16)
    c = np.asarray(ln_k_b, np.float32) @ Wkv  # [2*INNER]
    c_v = c[INNER:]
    Wout = np.asarray(Wout, np.float32)
    r = c_v @ Wout  # [D_MODEL]
    rrep = np.ascontiguousarray(np.broadcast_to(r, (128, D_MODEL))).astype(np.float32)
    wout_arr = np.ascontiguousarray(
        Wout.reshape(N_HEAD, DH, D_MODEL).transpose(1, 0, 2)
    ).astype(ml_dtypes.bfloat16)
    return {"wp": Wp, "qt": qT, "wout": wout_arr, "rrep": rrep}


_NC_CACHE = {}


def get_nc():
    if "nc" not in _NC_CACHE:
        _NC_CACHE["nc"] = build_nc()
    return _NC_CACHE["nc"]


def kernel(x, query, ln_q_w, ln_q_b, ln_k_w, ln_k_b, Wq, Wkv, Wout):
    x = np.asarray(x, np.float32)
    shared = host_prep(query, ln_q_w, ln_q_b, ln_k_w, ln_k_b, Wq, Wkv, Wout)
    in_maps = [
        {"x": np.ascontiguousarray(x[b]).astype(ml_dtypes.bfloat16), **shared}
        for b in range(B)
    ]
    nc = get_nc()
    res = run_bass_kernel_spmd(nc, in_maps, list(range(N_CORES)))
    return np.stack([res.results[b]["out"] for b in range(B)], axis=0)
